# revision 6
# baseline (speedup 1.0000x reference)
"""Trainium2 Bass kernel for nn_Attention_65541200937161 (sparse_attention).

Computation (per batch b, head h; B=16, N=1024, E=512, H=8, DH=64):
    qh = (q @ Wq.T + bq) split heads;  kh, vh same
    att = softmax(qh @ kh.T / sqrt(DH) + d) * d
    out = (att @ vh merged heads) @ Wp.T + bp

Sharding: data-parallel over batch B across 8 cores (2 batches/core).
Layout strategy per core (transposed-scores):
    - inputs/weights transposed on PE (fp32), cast to bf16 at PSUM evac
    - QT/KT [dh, tokens] per head-pair tile; VP natural [tokens, f]
    - S^T[m,n] = KT_h^T-slice.T @ QT_h-slice  (K=64 single matmuls)
    - +d: identity-matmul accumulate of dT (bf16) onto the same PSUM
    - exp on ACT PSUM->SBUF (bf16 out)
    - row-sums Sum_m e[m,n] via ones-column matmul (PE)
    - att'' = e * dT on DVE (bf16 2x)
    - x^T[dh,n] = sum_m VP[m,dh].T @ att''[m,n] accumulated on PE
    - normalize x^T by 1/s[n]: reciprocal (DVE), broadcast via K=1 f32r
      ones-matmul, multiply at PSUM evac into XT (bf16)
    - out = XT.T @ WpT + bp, natural layout, DMA out fp32
The scale 1/sqrt(DH) is folded into Wq/bq on the host.
"""

import math

import numpy as np

import concourse.bass as bass
import concourse.tile as tile
from concourse import bacc, mybir
from concourse.masks import make_identity

P = 128
E = 512
N = 1024
H = 8
DH = 64
B = 16
NCORES = 8
BLOC = B // NCORES          # 2 batches per core
NT = BLOC * N               # 2048 tokens per core

F32 = mybir.dt.float32
F32R = mybir.dt.float32r
BF16 = mybir.dt.bfloat16
EXP = mybir.ActivationFunctionType.Exp
MULT = mybir.AluOpType.mult

_CACHE = {}


def _build_nc():
    nc = bacc.Bacc("TRN2", target_bir_lowering=False, debug=False, num_devices=1)

    dq = nc.dram_tensor("q", [NT, E], F32, kind="ExternalInput")
    dk = nc.dram_tensor("k", [NT, E], F32, kind="ExternalInput")
    dv = nc.dram_tensor("v", [NT, E], F32, kind="ExternalInput")
    dd = nc.dram_tensor("d", [NT, N], F32, kind="ExternalInput")
    dW = [nc.dram_tensor(f"W{s}", [E, E], F32, kind="ExternalInput")
          for s in "qkvp"]
    db = [nc.dram_tensor(f"b{s}", [1, E], F32, kind="ExternalInput")
          for s in "qkvp"]
    dout = nc.dram_tensor("out", [NT, E], F32, kind="ExternalOutput")

    with tile.TileContext(nc) as tc:
        _emit(nc, tc, dq, dk, dv, dd, dW, db, dout)
    nc.compile()
    return nc


def _emit(nc, tc, dq, dk, dv, dd, dW, db, dout):
    from contextlib import ExitStack

    with ExitStack() as ctx:
        const = ctx.enter_context(tc.tile_pool(name="const", bufs=1))
        persist = ctx.enter_context(tc.tile_pool(name="persist", bufs=1))

        # ---- constants ----
        ident = const.tile([P, P], F32, tag="ident")
        make_identity(nc, ident[:])
        ident_bf = const.tile([P, P], BF16, tag="ident_bf")
        nc.vector.tensor_copy(ident_bf[:], ident[:])
        ones_col = const.tile([P, 1], BF16, tag="ones_col")
        nc.vector.memset(ones_col[:], 1.0)
        ones_row = const.tile([1, E], BF16, tag="ones_row")
        nc.vector.memset(ones_row[:], 1.0)
        ones64f = const.tile([1, DH], F32, tag="ones64f")
        nc.vector.memset(ones64f[:], 1.0)
        ones64r = const.tile([1, DH], F32R, tag="ones64r")
        nc.vector.tensor_copy(ones64r[:], ones64f[:])

        # biases as bf16 rows (K=1 matmul operands)
        b_bf = []
        for i in range(4):
            braw = const.tile([1, E], F32, tag=f"braw{i}")
            nc.sync.dma_start(braw[:], db[i].ap())
            bb = const.tile([1, E], BF16, tag=f"bbf{i}")
            nc.vector.tensor_copy(bb[:], braw[:])
            b_bf.append(bb)

        # ---- persistent SBUF tensors ----
        # WT[w][e]: [128e, 512f] bf16  (W^T tiles)
        WT = [[persist.tile([P, E], BF16, tag=f"wt{w}_{e}", name=f"wt{w}_{e}") for e in range(4)]
              for w in range(4)]
        # QT/KT: 4 tiles [128f, 2048t] bf16; heads 2ft,2ft+1 at part 0/64
        QT = [persist.tile([P, NT], BF16, tag=f"qt{i}", name=f"qt{i}") for i in range(4)]
        KT = [persist.tile([P, NT], BF16, tag=f"kt{i}", name=f"kt{i}") for i in range(4)]
        # VP: 16 tiles [128t, 512f] bf16 (projected v, natural layout)
        VP = [persist.tile([P, E], BF16, tag=f"vp{i}", name=f"vp{i}") for i in range(16)]
        # dT_bf[b][m]: [128m, 1024n] bf16 (d transposed per batch)
        DT = [[persist.tile([P, N], BF16, tag=f"dt{b}_{m}", name=f"dt{b}_{m}") for m in range(8)]
              for b in range(BLOC)]
        # XT: 4 tiles [128e, 2048t] bf16 (attention output, transposed)
        XT = [persist.tile([P, NT], BF16, tag=f"xtout{i}", name=f"xtout{i}") for i in range(4)]

        # ================= phase W + I + D: transposes & projections ========
        with ExitStack() as pctx:
            stage = pctx.enter_context(tc.tile_pool(name="stage", bufs=3))
            xtin = pctx.enter_context(tc.tile_pool(name="xtin", bufs=1))
            ptr_pool = pctx.enter_context(
                tc.tile_pool(name="ptr", bufs=2, space="PSUM"))
            pj_pool = pctx.enter_context(
                tc.tile_pool(name="pj", bufs=2, space="PSUM"))
            dtr_pool = pctx.enter_context(
                tc.tile_pool(name="dtr", bufs=4, space="PSUM"))

            # ---- weights: load, transpose, cast ----
            for w in range(4):
                wraw = stage.tile([P, 4 * E], F32, tag="raw", name="wraw")
                nc.sync.dma_start(
                    wraw[:].rearrange("p (a e) -> p a e", a=4),
                    dW[w].ap().rearrange("(a p) e -> p a e", p=P))
                for e in range(4):
                    ptr = ptr_pool.tile([P, E], F32, tag="tr")
                    for a in range(4):
                        nc.tensor.transpose(
                            ptr[:, a * P:(a + 1) * P],
                            wraw[:, a * E + e * P: a * E + (e + 1) * P],
                            ident[:])
                    nc.scalar.copy(WT[w][e][:], ptr[:])

            # ---- q, k: transpose input, project to QT/KT ----
            xt_in = [xtin.tile([P, NT], BF16, tag=f"xtin{e}", name=f"xtin{e}") for e in range(4)]
            for w, (dx, QKT) in enumerate([(dq, QT), (dk, KT)]):
                for c in range(4):
                    inraw = stage.tile([P, 4 * E], F32, tag="raw", name="inraw")
                    nc.sync.dma_start(
                        inraw[:].rearrange("p (a e) -> p a e", a=4),
                        dx.ap()[c * 512:(c + 1) * 512, :]
                        .rearrange("(a p) e -> p a e", p=P))
                    for e in range(4):
                        ptr = ptr_pool.tile([P, E], F32, tag="tr")
                        for a in range(4):
                            nc.tensor.transpose(
                                ptr[:, a * P:(a + 1) * P],
                                inraw[:, a * E + e * P: a * E + (e + 1) * P],
                                ident[:])
                        nc.scalar.copy(
                            xt_in[e][:, c * E:(c + 1) * E], ptr[:])
                for ft in range(4):
                    for c in range(4):
                        pp = pj_pool.tile([P, E], F32, tag="pj")
                        for e in range(4):
                            nc.tensor.matmul(
                                pp[:],
                                WT[w][e][:, ft * P:(ft + 1) * P],
                                xt_in[e][:, c * E:(c + 1) * E],
                                start=(e == 0), stop=(e == 3))
                        # +bias[f] per partition: lhsT=bias slice, rhs=ones
                        nc.tensor.matmul(
                            pp[:], b_bf[w][:, ft * P:(ft + 1) * P],
                            ones_row[:], start=False, stop=True,
                            skip_group_check=True)
                        nc.scalar.copy(QKT[ft][:, c * E:(c + 1) * E], pp[:])

            # ---- v: transpose input, project to VP (natural) ----
            for c in range(4):
                inraw = stage.tile([P, 4 * E], F32, tag="raw", name="inraw")
                nc.sync.dma_start(
                    inraw[:].rearrange("p (a e) -> p a e", a=4),
                    dv.ap()[c * 512:(c + 1) * 512, :]
                    .rearrange("(a p) e -> p a e", p=P))
                for e in range(4):
                    ptr = ptr_pool.tile([P, E], F32, tag="tr")
                    for a in range(4):
                        nc.tensor.transpose(
                            ptr[:, a * P:(a + 1) * P],
                            inraw[:, a * E + e * P: a * E + (e + 1) * P],
                            ident[:])
                    nc.scalar.copy(xt_in[e][:, c * E:(c + 1) * E], ptr[:])
            for t in range(16):
                pp = pj_pool.tile([P, E], F32, tag="pj")
                for e in range(4):
                    nc.tensor.matmul(
                        pp[:],
                        xt_in[e][:, t * P:(t + 1) * P],
                        WT[2][e][:],
                        start=(e == 0), stop=(e == 3))
                nc.tensor.matmul(
                    pp[:], ones_row[:, 0:P], b_bf[2][:],
                    start=False, stop=True, skip_group_check=True)
                nc.scalar.copy(VP[t][:], pp[:])

            # ---- d: transpose per batch ----
            for b in range(BLOC):
                for c in range(4):   # chunks of 2 n-tiles
                    draw = stage.tile([P, 2 * N], F32, tag="raw", name="draw")
                    r0 = b * N + c * 256
                    nc.sync.dma_start(
                        draw[:].rearrange("p (a m) -> p a m", a=2),
                        dd.ap()[r0:r0 + 256, :]
                        .rearrange("(a p) m -> p a m", p=P))
                    for m in range(8):
                        ptr = dtr_pool.tile([P, 256], F32, tag="dtr")
                        for a in range(2):
                            nc.tensor.transpose(
                                ptr[:, a * P:(a + 1) * P],
                                draw[:, a * N + m * P: a * N + (m + 1) * P],
                                ident[:])
                        nc.vector.tensor_copy(
                            DT[b][m][:, c * 256:(c + 1) * 256], ptr[:])

        # ================= phase A: attention ==============================
        with ExitStack() as actx:
            att_ps = actx.enter_context(
                tc.tile_pool(name="attps", bufs=1, space="PSUM"))
            asb = actx.enter_context(tc.tile_pool(name="asb", bufs=3))
            nsb = actx.enter_context(tc.tile_pool(name="nsb", bufs=2))

            for b in range(BLOC):
                for h in range(H):
                    ft, po = h // 2, (h % 2) * DH
                    s_ps = att_ps.tile([1, N], F32, tag="s")
                    x_ps = att_ps.tile([DH, N], F32, tag="x")
                    for m in range(8):
                        st = att_ps.tile([P, N], F32, tag="st", bufs=2)
                        for ch in range(2):
                            cs = slice(ch * 512, (ch + 1) * 512)
                            nc.tensor.matmul(
                                st[:, cs],
                                KT[ft][po:po + DH,
                                       b * N + m * P: b * N + (m + 1) * P],
                                QT[ft][po:po + DH,
                                       b * N + ch * 512: b * N + (ch + 1) * 512],
                                start=True, stop=True)
                            nc.tensor.matmul(
                                st[:, cs], ident_bf[:], DT[b][m][:, cs],
                                start=False, stop=True, skip_group_check=True)
                        e_t = asb.tile([P, N], BF16, tag="e")
                        nc.scalar.activation(e_t[:], st[:], EXP)
                        for ch in range(2):
                            cs = slice(ch * 512, (ch + 1) * 512)
                            nc.tensor.matmul(
                                s_ps[:, cs], ones_col[:], e_t[:, cs],
                                start=(m == 0), stop=(m == 7))
                        att_t = asb.tile([P, N], BF16, tag="att")
                        nc.vector.tensor_tensor(
                            att_t[:], e_t[:], DT[b][m][:], MULT)
                        for ch in range(2):
                            cs = slice(ch * 512, (ch + 1) * 512)
                            nc.tensor.matmul(
                                x_ps[:, cs],
                                VP[b * 8 + m][:, h * DH:(h + 1) * DH],
                                att_t[:, cs],
                                start=(m == 0), stop=(m == 7))
                    # normalize: XT slice = x_ps * (1/s) broadcast
                    r_f = nsb.tile([1, N], F32, tag="r")
                    nc.vector.reciprocal(r_f[:], s_ps[:])
                    r_r = nsb.tile([1, N], F32R, tag="rr")
                    nc.vector.tensor_copy(r_r[:], r_f[:])
                    rb = att_ps.tile([DH, N], F32, tag="st", bufs=2)
                    for ch in range(2):
                        cs = slice(ch * 512, (ch + 1) * 512)
                        nc.tensor.matmul(
                            rb[:, cs], ones64r[:], r_r[:, cs],
                            start=True, stop=True)
                    rb_sb = nsb.tile([DH, N], F32, tag="rb")
                    nc.scalar.copy(rb_sb[:], rb[:])
                    nc.vector.tensor_tensor(
                        XT[ft][po:po + DH, b * N:(b + 1) * N],
                        x_ps[:], rb_sb[:], MULT)

            # ---- output projection ----
            for t in range(16):
                pp = att_ps.tile([P, E], F32, tag="st", bufs=2)
                for e in range(4):
                    nc.tensor.matmul(
                        pp[:], XT[e][:, t * P:(t + 1) * P], WT[3][e][:],
                        start=(e == 0), stop=(e == 3))
                nc.tensor.matmul(
                    pp[:], ones_row[:, 0:P], b_bf[3][:],
                    start=False, stop=True, skip_group_check=True)
                osb = asb.tile([P, E], F32, tag="osb")
                nc.scalar.copy(osb[:], pp[:])
                nc.sync.dma_start(dout.ap()[t * P:(t + 1) * P, :], osb[:])


def _get_nc():
    if "nc" not in _CACHE:
        _CACHE["nc"] = _build_nc()
    return _CACHE["nc"]


def _shard(inputs):
    q, k, v, d = (np.asarray(inputs[s], np.float32) for s in "qkvd")
    scale = 1.0 / math.sqrt(DH)
    Wq = np.asarray(inputs["Wq"], np.float32) * scale
    bq = np.asarray(inputs["bq"], np.float32) * scale
    Ws = [Wq, np.asarray(inputs["Wk"], np.float32),
          np.asarray(inputs["Wv"], np.float32),
          np.asarray(inputs["Wp"], np.float32)]
    bs = [bq, np.asarray(inputs["bk"], np.float32),
          np.asarray(inputs["bv"], np.float32),
          np.asarray(inputs["bp"], np.float32)]
    in_maps = []
    for c in range(NCORES):
        sl = slice(c * BLOC, (c + 1) * BLOC)
        m = {
            "q": np.ascontiguousarray(q[sl].reshape(NT, E)),
            "k": np.ascontiguousarray(k[sl].reshape(NT, E)),
            "v": np.ascontiguousarray(v[sl].reshape(NT, E)),
            "d": np.ascontiguousarray(d[sl].reshape(NT, N)),
        }
        for i, s in enumerate("qkvp"):
            m[f"W{s}"] = np.ascontiguousarray(Ws[i])
            m[f"b{s}"] = np.ascontiguousarray(bs[i].reshape(1, E))
        in_maps.append(m)
    return in_maps


def kernel(**inputs):
    from concourse.bass_utils import run_bass_kernel_spmd
    nc = _get_nc()
    in_maps = _shard(inputs)
    res = run_bass_kernel_spmd(nc, in_maps, core_ids=list(range(NCORES)))
    outs = [res.results[c]["out"].reshape(BLOC, N, E) for c in range(NCORES)]
    return np.concatenate(outs, axis=0)


# revision 9
# speedup vs baseline: 53.3539x; 53.3539x over previous
"""Trainium2 Bass kernel for nn_Attention_65541200937161 (sparse_attention).

Computation (per batch b, head h; B=16, N=1024, E=512, H=8, DH=64):
    qh = (q @ Wq.T + bq) split heads;  kh, vh same
    att = softmax(qh @ kh.T / sqrt(DH) + d) * d
    out = (att @ vh merged heads) @ Wp.T + bp

Sharding: data-parallel over batch B across 8 cores (2 batches/core).
Layout strategy per core (transposed-scores):
    - inputs/weights transposed on PE (fp32), cast to bf16 at PSUM evac
    - QT/KT [dh, tokens] per head-pair tile; VP natural [tokens, f]
    - S^T[m,n] = KT_h^T-slice.T @ QT_h-slice  (K=64 single matmuls)
    - +d: identity-matmul accumulate of dT (bf16) onto the same PSUM
    - exp on ACT PSUM->SBUF (bf16 out)
    - row-sums Sum_m e[m,n] via ones-column matmul (PE)
    - att'' = e * dT on DVE (bf16 2x)
    - x^T[dh,n] = sum_m VP[m,dh].T @ att''[m,n] accumulated on PE
    - normalize x^T by 1/s[n]: reciprocal (DVE), broadcast via K=1 f32r
      ones-matmul, multiply at PSUM evac into XT (bf16)
    - out = XT.T @ WpT + bp, natural layout, DMA out fp32
The scale 1/sqrt(DH) is folded into Wq/bq on the host.
"""

import math

import numpy as np

import concourse.bass as bass
import concourse.tile as tile
from concourse import bacc, mybir
from concourse.masks import make_identity

P = 128
E = 512
N = 1024
H = 8
DH = 64
B = 16
NCORES = 8
BLOC = B // NCORES          # 2 batches per core
NT = BLOC * N               # 2048 tokens per core

F32 = mybir.dt.float32
F32R = mybir.dt.float32r
BF16 = mybir.dt.bfloat16
EXP = mybir.ActivationFunctionType.Exp
MULT = mybir.AluOpType.mult

_CACHE = {}


def _build_nc():
    nc = bacc.Bacc("TRN2", target_bir_lowering=False, debug=False, num_devices=1)

    dq = nc.dram_tensor("q", [NT, E], F32, kind="ExternalInput")
    dk = nc.dram_tensor("k", [NT, E], F32, kind="ExternalInput")
    dv = nc.dram_tensor("v", [NT, E], F32, kind="ExternalInput")
    dd = nc.dram_tensor("d", [NT, N], F32, kind="ExternalInput")
    dW = [nc.dram_tensor(f"W{s}", [E, E], F32, kind="ExternalInput")
          for s in "qkvp"]
    db = [nc.dram_tensor(f"b{s}", [1, E], F32, kind="ExternalInput")
          for s in "qkvp"]
    dout = nc.dram_tensor("out", [NT, E], F32, kind="ExternalOutput")

    with tile.TileContext(nc) as tc:
        _emit(nc, tc, dq, dk, dv, dd, dW, db, dout)
    nc.compile()
    return nc


def _emit(nc, tc, dq, dk, dv, dd, dW, db, dout):
    from contextlib import ExitStack

    with ExitStack() as ctx:
        const = ctx.enter_context(tc.tile_pool(name="const", bufs=1))
        persist = ctx.enter_context(tc.tile_pool(name="persist", bufs=1))

        # ---- constants ----
        ident = const.tile([P, P], F32, tag="ident")
        make_identity(nc, ident[:])
        ident_bf = const.tile([P, P], BF16, tag="ident_bf")
        nc.vector.tensor_copy(ident_bf[:], ident[:])
        ones_col = const.tile([P, 1], BF16, tag="ones_col")
        nc.vector.memset(ones_col[:], 1.0)
        ones_row = const.tile([1, E], BF16, tag="ones_row")
        nc.vector.memset(ones_row[:], 1.0)
        ones64f = const.tile([1, DH], F32, tag="ones64f")
        nc.vector.memset(ones64f[:], 1.0)
        ones64r = const.tile([1, DH], F32R, tag="ones64r")
        nc.vector.tensor_copy(ones64r[:], ones64f[:])

        # biases as bf16 rows (K=1 matmul operands)
        b_bf = []
        for i in range(4):
            braw = const.tile([1, E], F32, tag=f"braw{i}")
            nc.sync.dma_start(braw[:], db[i].ap())
            bb = const.tile([1, E], BF16, tag=f"bbf{i}")
            nc.vector.tensor_copy(bb[:], braw[:])
            b_bf.append(bb)

        # ---- persistent SBUF tensors ----
        # WT[w][e]: [128e, 512f] bf16  (W^T tiles)
        WT = [[persist.tile([P, E], BF16, tag=f"wt{w}_{e}", name=f"wt{w}_{e}") for e in range(4)]
              for w in range(4)]
        # QT/KT: 4 tiles [128f, 2048t] bf16; heads 2ft,2ft+1 at part 0/64
        QT = [persist.tile([P, NT], BF16, tag=f"qt{i}", name=f"qt{i}") for i in range(4)]
        KT = [persist.tile([P, NT], BF16, tag=f"kt{i}", name=f"kt{i}") for i in range(4)]
        # VP: 16 tiles [128t, 512f] bf16 (projected v, natural layout)
        VP = [persist.tile([P, E], BF16, tag=f"vp{i}", name=f"vp{i}") for i in range(16)]
        # dT_bf[b][m]: [128m, 1024n] bf16 (d transposed per batch)
        DT = [[persist.tile([P, N], BF16, tag=f"dt{b}_{m}", name=f"dt{b}_{m}") for m in range(8)]
              for b in range(BLOC)]
        # XT: 4 tiles [128e, 2048t] bf16 (attention output, transposed)
        XT = [persist.tile([P, NT], BF16, tag=f"xtout{i}", name=f"xtout{i}") for i in range(4)]

        # ================= phase W + I + D: transposes & projections ========
        with ExitStack() as pctx:
            stage = pctx.enter_context(tc.tile_pool(name="stage", bufs=3))
            xtin = pctx.enter_context(tc.tile_pool(name="xtin", bufs=1))
            ptr_pool = pctx.enter_context(
                tc.tile_pool(name="ptr", bufs=2, space="PSUM"))
            pj_pool = pctx.enter_context(
                tc.tile_pool(name="pj", bufs=2, space="PSUM"))
            dtr_pool = pctx.enter_context(
                tc.tile_pool(name="dtr", bufs=4, space="PSUM"))

            # ---- weights: load, transpose, cast ----
            for w in range(4):
                wraw = stage.tile([P, 4 * E], F32, tag="raw", name="wraw")
                nc.sync.dma_start(
                    wraw[:].rearrange("p (a e) -> p a e", a=4),
                    dW[w].ap().rearrange("(a p) e -> p a e", p=P))
                for e in range(4):
                    ptr = ptr_pool.tile([P, E], F32, tag="tr")
                    for a in range(4):
                        nc.tensor.transpose(
                            ptr[:, a * P:(a + 1) * P],
                            wraw[:, a * E + e * P: a * E + (e + 1) * P],
                            ident[:])
                    nc.scalar.copy(WT[w][e][:], ptr[:])

            # ---- q, k: transpose input, project to QT/KT ----
            xt_in = [xtin.tile([P, NT], BF16, tag=f"xtin{e}", name=f"xtin{e}") for e in range(4)]
            for w, (dx, QKT) in enumerate([(dq, QT), (dk, KT)]):
                for c in range(4):
                    inraw = stage.tile([P, 4 * E], F32, tag="raw", name="inraw")
                    nc.sync.dma_start(
                        inraw[:].rearrange("p (a e) -> p a e", a=4),
                        dx.ap()[c * 512:(c + 1) * 512, :]
                        .rearrange("(a p) e -> p a e", p=P))
                    for e in range(4):
                        ptr = ptr_pool.tile([P, E], F32, tag="tr")
                        for a in range(4):
                            nc.tensor.transpose(
                                ptr[:, a * P:(a + 1) * P],
                                inraw[:, a * E + e * P: a * E + (e + 1) * P],
                                ident[:])
                        nc.scalar.copy(
                            xt_in[e][:, c * E:(c + 1) * E], ptr[:])
                for ft in range(4):
                    for c in range(4):
                        pp = pj_pool.tile([P, E], F32, tag="pj")
                        for e in range(4):
                            nc.tensor.matmul(
                                pp[:],
                                WT[w][e][:, ft * P:(ft + 1) * P],
                                xt_in[e][:, c * E:(c + 1) * E],
                                start=(e == 0), stop=(e == 3))
                        # +bias[f] per partition: lhsT=bias slice, rhs=ones
                        nc.tensor.matmul(
                            pp[:], b_bf[w][:, ft * P:(ft + 1) * P],
                            ones_row[:], start=False, stop=True,
                            skip_group_check=True)
                        nc.scalar.copy(QKT[ft][:, c * E:(c + 1) * E], pp[:])

            # ---- v: transpose input, project to VP (natural) ----
            for c in range(4):
                inraw = stage.tile([P, 4 * E], F32, tag="raw", name="inraw")
                nc.sync.dma_start(
                    inraw[:].rearrange("p (a e) -> p a e", a=4),
                    dv.ap()[c * 512:(c + 1) * 512, :]
                    .rearrange("(a p) e -> p a e", p=P))
                for e in range(4):
                    ptr = ptr_pool.tile([P, E], F32, tag="tr")
                    for a in range(4):
                        nc.tensor.transpose(
                            ptr[:, a * P:(a + 1) * P],
                            inraw[:, a * E + e * P: a * E + (e + 1) * P],
                            ident[:])
                    nc.scalar.copy(xt_in[e][:, c * E:(c + 1) * E], ptr[:])
            for t in range(16):
                pp = pj_pool.tile([P, E], F32, tag="pj")
                for e in range(4):
                    nc.tensor.matmul(
                        pp[:],
                        xt_in[e][:, t * P:(t + 1) * P],
                        WT[2][e][:],
                        start=(e == 0), stop=(e == 3))
                nc.tensor.matmul(
                    pp[:], ones_row[:, 0:P], b_bf[2][:],
                    start=False, stop=True, skip_group_check=True)
                nc.scalar.copy(VP[t][:], pp[:])

            # ---- d: transpose per batch ----
            for b in range(BLOC):
                for c in range(4):   # chunks of 2 n-tiles
                    draw = stage.tile([P, 2 * N], F32, tag="raw", name="draw")
                    r0 = b * N + c * 256
                    nc.sync.dma_start(
                        draw[:].rearrange("p (a m) -> p a m", a=2),
                        dd.ap()[r0:r0 + 256, :]
                        .rearrange("(a p) m -> p a m", p=P))
                    for m in range(8):
                        ptr = dtr_pool.tile([P, 256], F32, tag="dtr")
                        for a in range(2):
                            nc.tensor.transpose(
                                ptr[:, a * P:(a + 1) * P],
                                draw[:, a * N + m * P: a * N + (m + 1) * P],
                                ident[:])
                        nc.vector.tensor_copy(
                            DT[b][m][:, c * 256:(c + 1) * 256], ptr[:])

        # ================= phase A: attention ==============================
        with ExitStack() as actx:
            att_ps = actx.enter_context(
                tc.tile_pool(name="attps", bufs=1, space="PSUM"))
            asb = actx.enter_context(tc.tile_pool(name="asb", bufs=3))
            nsb = actx.enter_context(tc.tile_pool(name="nsb", bufs=2))

            for b in range(BLOC):
                for h in range(H):
                    ft, po = h // 2, (h % 2) * DH
                    s_ps = att_ps.tile([1, N], F32, tag="s")
                    x_ps = att_ps.tile([DH, N], F32, tag="x")
                    for m in range(8):
                        st = att_ps.tile([P, N], F32, tag="st", bufs=2)
                        for ch in range(2):
                            cs = slice(ch * 512, (ch + 1) * 512)
                            nc.tensor.matmul(
                                st[:, cs],
                                KT[ft][po:po + DH,
                                       b * N + m * P: b * N + (m + 1) * P],
                                QT[ft][po:po + DH,
                                       b * N + ch * 512: b * N + (ch + 1) * 512],
                                start=True, stop=True)
                            nc.tensor.matmul(
                                st[:, cs], ident_bf[:], DT[b][m][:, cs],
                                start=False, stop=True, skip_group_check=True)
                        e_t = asb.tile([P, N], BF16, tag="e")
                        nc.scalar.activation(e_t[:], st[:], EXP)
                        for ch in range(2):
                            cs = slice(ch * 512, (ch + 1) * 512)
                            nc.tensor.matmul(
                                s_ps[:, cs], ones_col[:], e_t[:, cs],
                                start=(m == 0), stop=(m == 7))
                        att_t = asb.tile([P, N], BF16, tag="att")
                        nc.vector.tensor_tensor(
                            att_t[:], e_t[:], DT[b][m][:], MULT)
                        for ch in range(2):
                            cs = slice(ch * 512, (ch + 1) * 512)
                            nc.tensor.matmul(
                                x_ps[:, cs],
                                VP[b * 8 + m][:, h * DH:(h + 1) * DH],
                                att_t[:, cs],
                                start=(m == 0), stop=(m == 7))
                    # normalize: XT slice = x_ps * (1/s) broadcast
                    r_f = nsb.tile([1, N], F32, tag="r")
                    nc.vector.reciprocal(r_f[:], s_ps[:])
                    r_r = nsb.tile([1, N], F32R, tag="rr")
                    nc.vector.tensor_copy(r_r[:], r_f[:])
                    rb = att_ps.tile([DH, N], F32, tag="st", bufs=2)
                    for ch in range(2):
                        cs = slice(ch * 512, (ch + 1) * 512)
                        nc.tensor.matmul(
                            rb[:, cs], ones64r[:], r_r[:, cs],
                            start=True, stop=True)
                    rb_sb = nsb.tile([DH, N], F32, tag="rb")
                    nc.scalar.copy(rb_sb[:], rb[:])
                    nc.vector.tensor_tensor(
                        XT[ft][po:po + DH, b * N:(b + 1) * N],
                        x_ps[:], rb_sb[:], MULT)

            # ---- output projection ----
            for t in range(16):
                pp = att_ps.tile([P, E], F32, tag="st", bufs=2)
                for e in range(4):
                    nc.tensor.matmul(
                        pp[:], XT[e][:, t * P:(t + 1) * P], WT[3][e][:],
                        start=(e == 0), stop=(e == 3))
                nc.tensor.matmul(
                    pp[:], ones_row[:, 0:P], b_bf[3][:],
                    start=False, stop=True, skip_group_check=True)
                osb = asb.tile([P, E], F32, tag="osb")
                nc.scalar.copy(osb[:], pp[:])
                nc.sync.dma_start(dout.ap()[t * P:(t + 1) * P, :], osb[:])


def _get_nc():
    if "nc" not in _CACHE:
        _CACHE["nc"] = _build_nc()
    return _CACHE["nc"]


def _shard(inputs):
    q, k, v, d = (np.asarray(inputs[s], np.float32) for s in "qkvd")
    scale = 1.0 / math.sqrt(DH)
    Wq = np.asarray(inputs["Wq"], np.float32) * scale
    bq = np.asarray(inputs["bq"], np.float32) * scale
    Ws = [Wq, np.asarray(inputs["Wk"], np.float32),
          np.asarray(inputs["Wv"], np.float32),
          np.asarray(inputs["Wp"], np.float32)]
    bs = [bq, np.asarray(inputs["bk"], np.float32),
          np.asarray(inputs["bv"], np.float32),
          np.asarray(inputs["bp"], np.float32)]
    in_maps = []
    for c in range(NCORES):
        sl = slice(c * BLOC, (c + 1) * BLOC)
        m = {
            "q": np.ascontiguousarray(q[sl].reshape(NT, E)),
            "k": np.ascontiguousarray(k[sl].reshape(NT, E)),
            "v": np.ascontiguousarray(v[sl].reshape(NT, E)),
            "d": np.ascontiguousarray(d[sl].reshape(NT, N)),
        }
        for i, s in enumerate("qkvp"):
            m[f"W{s}"] = np.ascontiguousarray(Ws[i])
            m[f"b{s}"] = np.ascontiguousarray(bs[i].reshape(1, E))
        in_maps.append(m)
    return in_maps


def _get_exec():
    """Build (once) a sharded jitted callable over the 8 axon devices."""
    if "exec" in _CACHE:
        return _CACHE["exec"]
    import jax
    from jax.sharding import Mesh, NamedSharding, PartitionSpec
    from jax.experimental.shard_map import shard_map
    from concourse import bass2jax

    nc = _get_nc()
    bass2jax.install_neuronx_cc_hook()

    partition_name = (nc.partition_id_tensor.name
                      if nc.partition_id_tensor else None)
    in_names, out_names, out_avals, zero_outs = [], [], [], []
    for alloc in nc.m.functions[0].allocations:
        if not isinstance(alloc, mybir.MemoryLocationSet):
            continue
        name = alloc.memorylocations[0].name
        if alloc.kind == "ExternalInput":
            if name != partition_name:
                in_names.append(name)
        elif alloc.kind == "ExternalOutput":
            out_names.append(name)
            shape = tuple(alloc.tensor_shape)
            dtype = mybir.dt.np(alloc.dtype)
            out_avals.append(jax.core.ShapedArray(shape, dtype))
            zero_outs.append(np.zeros(shape, dtype))
    n_params = len(in_names)
    all_names = in_names + out_names
    if partition_name is not None:
        all_names = all_names + [partition_name]

    def _body(*args):
        operands = list(args)
        if partition_name is not None:
            operands.append(bass2jax.partition_id_tensor())
        outs = bass2jax._bass_exec_p.bind(
            *operands,
            out_avals=tuple(out_avals),
            in_names=tuple(all_names),
            out_names=tuple(out_names),
            lowering_input_output_aliases=(),
            sim_require_finite=True,
            sim_require_nnan=True,
            nc=nc,
        )
        return tuple(outs)

    devices = jax.devices()[:NCORES]
    mesh = Mesh(np.asarray(devices), ("core",))
    nspec = (PartitionSpec("core"),)
    fn = jax.jit(
        shard_map(_body, mesh=mesh,
                  in_specs=nspec * (n_params + len(out_names)),
                  out_specs=nspec * len(out_names), check_rep=False),
        keep_unused=True)
    sharding = NamedSharding(mesh, PartitionSpec("core"))
    _CACHE["exec"] = (fn, in_names, out_names, out_avals, zero_outs, sharding)
    return _CACHE["exec"]


def _concat_args(in_maps):
    fn, in_names, out_names, out_avals, zero_outs, _ = _get_exec()
    concat_in = [
        np.concatenate([in_maps[c][nm] for c in range(NCORES)], axis=0)
        for nm in in_names]
    concat_zero = [
        np.zeros((NCORES * z.shape[0], *z.shape[1:]), z.dtype)
        for z in zero_outs]
    return concat_in + concat_zero


def kernel(**inputs):
    fn, in_names, out_names, out_avals, zero_outs, _ = _get_exec()
    args = _concat_args(_shard(inputs))
    out_arrs = fn(*args)
    out = np.asarray(out_arrs[out_names.index("out")])
    return out.reshape(B, N, E)


def bench(inputs, iters=10):
    """Time repeated executions with device-resident inputs; returns secs."""
    import time
    import jax
    fn, in_names, out_names, out_avals, zero_outs, sharding = _get_exec()
    args = _concat_args(_shard(inputs))
    dev_args = [jax.device_put(a, sharding) for a in args]
    jax.block_until_ready(dev_args)
    out = fn(*dev_args)          # warm
    jax.block_until_ready(out)
    times = []
    for _ in range(iters):
        t0 = time.perf_counter()
        out = fn(*dev_args)
        jax.block_until_ready(out)
        times.append(time.perf_counter() - t0)
    return times


# revision 14
# speedup vs baseline: 53.6030x; 1.0047x over previous
"""Trainium2 Bass kernel for nn_Attention_65541200937161 (sparse_attention).

Computation (per batch b, head h; B=16, N=1024, E=512, H=8, DH=64):
    qh = (q @ Wq.T + bq) split heads;  kh, vh same
    att = softmax(qh @ kh.T / sqrt(DH) + d) * d
    out = (att @ vh merged heads) @ Wp.T + bp

Sharding: data-parallel over batch B across 8 cores (2 batches/core).

Per-core layout (transposed-scores, head-paired):
    - inputs/weights cast to bf16 (DVE) then transposed on PE; evac at PSUM
    - QT/KT [128f, 2048t] bf16: head-pair 2ft,2ft+1 at partitions 0/64
    - attention loop over (batch, head-pair, n-half): K=64 QK matmuls for
      the two heads sit on disjoint PE row-groups (concurrent); +d via
      identity-matmul accumulate; exp on ACT (PSUM->SBUF, bf16);
      row-sums via ones-column matmuls into a shared [33,512] psum tile at
      partitions 0/32 (disjoint col-groups, concurrent); att''=e*dT on DVE;
      AV col-packed into x_pair [128,512] at partitions 0/64 (concurrent)
    - normalize by 1/rowsum: DVE reciprocal, f32r K=1 broadcast matmul,
      one DVE multiply into XT (bf16)
    - out = XT.T @ WpT + bp, natural layout, fp32 DMA out
The scale 1/sqrt(DH) is folded into Wq/bq on the host.
"""

import math
import os

import numpy as np

import concourse.bass as bass
import concourse.tile as tile
from concourse import bacc, mybir
from concourse.masks import make_identity

P = 128
E = 512
N = 1024
H = 8
DH = 64
B = 16
NCORES = 8
BLOC = B // NCORES          # 2 batches per core
NT = BLOC * N               # 2048 tokens per core

F32 = mybir.dt.float32
F32R = mybir.dt.float32r
BF16 = mybir.dt.bfloat16
EXP = mybir.ActivationFunctionType.Exp
MULT = mybir.AluOpType.mult

_CACHE = {}


def _build_nc(with_bias):
    repeat = int(os.environ.get("KERNEL_REPEAT", "1"))
    nc = bacc.Bacc("TRN2", target_bir_lowering=False, debug=False,
                   num_devices=1)

    dq = nc.dram_tensor("q", [NT, E], F32, kind="ExternalInput")
    dk = nc.dram_tensor("k", [NT, E], F32, kind="ExternalInput")
    dv = nc.dram_tensor("v", [NT, E], F32, kind="ExternalInput")
    dd = nc.dram_tensor("d", [NT, N], F32, kind="ExternalInput")
    dW = [nc.dram_tensor(f"W{s}", [E, E], F32, kind="ExternalInput")
          for s in "qkvp"]
    db = [nc.dram_tensor(f"b{s}", [1, E], F32, kind="ExternalInput")
          for s in "qkvp"]
    dout = nc.dram_tensor("out", [NT, E], F32, kind="ExternalOutput")

    with tile.TileContext(nc) as tc:
        for _ in range(repeat):
            _emit(nc, tc, dq, dk, dv, dd, dW, db, dout, with_bias)
    nc.compile()
    return nc


def _emit(nc, tc, dq, dk, dv, dd, dW, db, dout, with_bias):
    from contextlib import ExitStack

    with ExitStack() as ctx:
        const = ctx.enter_context(tc.tile_pool(name="const", bufs=1))
        persist = ctx.enter_context(tc.tile_pool(name="persist", bufs=1))

        # ---- constants ----
        ident = const.tile([P, P], F32, tag="ident")
        make_identity(nc, ident[:])
        ident_bf = const.tile([P, P], BF16, tag="ident_bf")
        nc.vector.tensor_copy(ident_bf[:], ident[:])
        ones_col = const.tile([P, 1], BF16, tag="ones_col")
        nc.vector.memset(ones_col[:], 1.0)
        ones64f = const.tile([1, DH], F32, tag="ones64f")
        nc.vector.memset(ones64f[:], 1.0)
        ones64r = const.tile([1, DH], F32R, tag="ones64r")
        nc.vector.tensor_copy(ones64r[:], ones64f[:])

        b_bf = []
        if with_bias:
            ones_row = const.tile([1, E], BF16, tag="ones_row")
            nc.vector.memset(ones_row[:], 1.0)
            for i in range(4):
                braw = const.tile([1, E], F32, tag=f"braw{i}")
                nc.sync.dma_start(braw[:], db[i].ap())
                bb = const.tile([1, E], BF16, tag=f"bbf{i}")
                nc.vector.tensor_copy(bb[:], braw[:])
                b_bf.append(bb)

        # ---- persistent SBUF tensors ----
        WT = [[persist.tile([P, E], BF16, tag=f"wt{w}_{e}", name=f"wt{w}_{e}")
               for e in range(4)] for w in range(4)]
        QT = [persist.tile([P, NT], BF16, tag=f"qt{i}", name=f"qt{i}")
              for i in range(4)]
        KT = [persist.tile([P, NT], BF16, tag=f"kt{i}", name=f"kt{i}")
              for i in range(4)]
        VP = [persist.tile([P, E], BF16, tag=f"vp{i}", name=f"vp{i}")
              for i in range(16)]
        DT = [[persist.tile([P, N], BF16, tag=f"dt{b}_{m}", name=f"dt{b}_{m}")
               for m in range(8)] for b in range(BLOC)]
        XT = [persist.tile([P, NT], BF16, tag=f"xtout{i}", name=f"xtout{i}")
              for i in range(4)]

        # ================= phase W/I/D: casts, transposes, projections ======
        with ExitStack() as pctx:
            stage = pctx.enter_context(tc.tile_pool(name="stage", bufs=2))
            bstage = pctx.enter_context(tc.tile_pool(name="bstage", bufs=3))
            xtin = pctx.enter_context(tc.tile_pool(name="xtin", bufs=1))
            ptr_pool = pctx.enter_context(
                tc.tile_pool(name="ptr", bufs=2, space="PSUM"))
            pj_pool = pctx.enter_context(
                tc.tile_pool(name="pj", bufs=2, space="PSUM"))
            dtr_pool = pctx.enter_context(
                tc.tile_pool(name="dtr", bufs=4, space="PSUM"))

            def load_cast(dram_ap, evac_engine):
                """DMA a [128, 2048] fp32 block, cast to bf16."""
                raw = stage.tile([P, 4 * E], F32, tag="raw", name="raw")
                nc.sync.dma_start(
                    raw[:].rearrange("p (a e) -> p a e", a=4), dram_ap)
                bfb = bstage.tile([P, 4 * E], BF16, tag="bfb", name="bfb")
                evac_engine(bfb[:], raw[:])
                return bfb

            # ---- weights ----
            for w in range(4):
                wbf = load_cast(
                    dW[w].ap().rearrange("(a p) e -> p a e", p=P),
                    nc.vector.tensor_copy)
                for e in range(4):
                    ptr = ptr_pool.tile([P, E], BF16, tag="tr")
                    for a in range(4):
                        nc.tensor.transpose(
                            ptr[:, a * P:(a + 1) * P],
                            wbf[:, a * E + e * P: a * E + (e + 1) * P],
                            ident_bf[:])
                    nc.scalar.copy(WT[w][e][:], ptr[:])

            # ---- q, k -> QT, KT ----
            xt_in = [xtin.tile([P, NT], BF16, tag=f"xtin{e}", name=f"xtin{e}")
                     for e in range(4)]
            for w, (dx, QKT) in enumerate([(dq, QT), (dk, KT)]):
                for c in range(4):
                    xbf = load_cast(
                        dx.ap()[c * 512:(c + 1) * 512, :]
                        .rearrange("(a p) e -> p a e", p=P),
                        nc.vector.tensor_copy)
                    for e in range(4):
                        ptr = ptr_pool.tile([P, E], BF16, tag="tr")
                        for a in range(4):
                            nc.tensor.transpose(
                                ptr[:, a * P:(a + 1) * P],
                                xbf[:, a * E + e * P: a * E + (e + 1) * P],
                                ident_bf[:])
                        eng = nc.scalar.copy if (e % 2) else \
                            (lambda o, i: nc.vector.tensor_copy(o, i))
                        eng(xt_in[e][:, c * E:(c + 1) * E], ptr[:])
                for ft in range(4):
                    for c in range(4):
                        pp = pj_pool.tile([P, E], F32, tag="pj")
                        for e in range(4):
                            nc.tensor.matmul(
                                pp[:],
                                WT[w][e][:, ft * P:(ft + 1) * P],
                                xt_in[e][:, c * E:(c + 1) * E],
                                start=(e == 0), stop=(e == 3))
                        if with_bias:
                            nc.tensor.matmul(
                                pp[:], b_bf[w][:, ft * P:(ft + 1) * P],
                                ones_row[:], start=False, stop=True,
                                skip_group_check=True)
                        if c % 2:
                            nc.scalar.copy(
                                QKT[ft][:, c * E:(c + 1) * E], pp[:])
                        else:
                            nc.vector.tensor_copy(
                                QKT[ft][:, c * E:(c + 1) * E], pp[:])

            # ---- v -> VP ----
            for c in range(4):
                xbf = load_cast(
                    dv.ap()[c * 512:(c + 1) * 512, :]
                    .rearrange("(a p) e -> p a e", p=P),
                    nc.vector.tensor_copy)
                for e in range(4):
                    ptr = ptr_pool.tile([P, E], BF16, tag="tr")
                    for a in range(4):
                        nc.tensor.transpose(
                            ptr[:, a * P:(a + 1) * P],
                            xbf[:, a * E + e * P: a * E + (e + 1) * P],
                            ident_bf[:])
                    eng = nc.scalar.copy if (e % 2) else \
                        (lambda o, i: nc.vector.tensor_copy(o, i))
                    eng(xt_in[e][:, c * E:(c + 1) * E], ptr[:])
            for t in range(16):
                pp = pj_pool.tile([P, E], F32, tag="pj")
                for e in range(4):
                    nc.tensor.matmul(
                        pp[:],
                        xt_in[e][:, t * P:(t + 1) * P],
                        WT[2][e][:],
                        start=(e == 0), stop=(e == 3))
                if with_bias:
                    nc.tensor.matmul(
                        pp[:], ones_row[:, 0:P], b_bf[2][:],
                        start=False, stop=True, skip_group_check=True)
                if t % 2:
                    nc.scalar.copy(VP[t][:], pp[:])
                else:
                    nc.vector.tensor_copy(VP[t][:], pp[:])

            # ---- d -> DT (transposed, bf16) ----
            for b in range(BLOC):
                for c in range(4):
                    r0 = b * N + c * 256
                    dbf = load_cast(
                        dd.ap()[r0:r0 + 256, :]
                        .rearrange("(a p) m -> p a m", p=P),
                        nc.vector.tensor_copy)
                    for m in range(8):
                        ptr = dtr_pool.tile([P, 256], BF16, tag="dtr")
                        for a in range(2):
                            nc.tensor.transpose(
                                ptr[:, a * P:(a + 1) * P],
                                dbf[:, a * N + m * P: a * N + (m + 1) * P],
                                ident_bf[:])
                        if m % 2:
                            nc.scalar.copy(
                                DT[b][m][:, c * 256:(c + 1) * 256], ptr[:])
                        else:
                            nc.vector.tensor_copy(
                                DT[b][m][:, c * 256:(c + 1) * 256], ptr[:])

        # ================= phase A: attention ==============================
        with ExitStack() as actx:
            att_ps = actx.enter_context(
                tc.tile_pool(name="attps", bufs=1, space="PSUM"))
            asb = actx.enter_context(tc.tile_pool(name="asb", bufs=3))
            nsb = actx.enter_context(tc.tile_pool(name="nsb", bufs=2))

            C = 512   # n-chunk
            for b in range(BLOC):
                for hp in range(4):
                    ft = hp
                    for nh in range(2):
                        ns0 = b * N + nh * C
                        s_pair = att_ps.tile([65, C], F32, tag="s", bufs=2)
                        x_pair = att_ps.tile([P, C], F32, tag="x", bufs=2)
                        for m in range(8):
                            mt = slice(b * N + m * P, b * N + (m + 1) * P)
                            nc_sl = slice(nh * C, (nh + 1) * C)
                            st0 = att_ps.tile([P, C], F32, tag="st", bufs=4)
                            st1 = att_ps.tile([P, C], F32, tag="st", bufs=4)
                            # QK for both heads: disjoint PE row-groups
                            nc.tensor.matmul(
                                st0[:], KT[ft][0:DH, mt],
                                QT[ft][0:DH, ns0:ns0 + C],
                                start=True, stop=True)
                            nc.tensor.matmul(
                                st1[:], KT[ft][DH:P, mt],
                                QT[ft][DH:P, ns0:ns0 + C],
                                start=True, stop=True)
                            # + d (identity matmul accumulate)
                            nc.tensor.matmul(
                                st0[:], ident_bf[:], DT[b][m][:, nc_sl],
                                start=False, stop=True, skip_group_check=True)
                            nc.tensor.matmul(
                                st1[:], ident_bf[:], DT[b][m][:, nc_sl],
                                start=False, stop=True, skip_group_check=True)
                            e0 = asb.tile([P, C], BF16, tag="e")
                            e1 = asb.tile([P, C], BF16, tag="e")
                            nc.scalar.activation(e0[:], st0[:], EXP)
                            nc.scalar.activation(e1[:], st1[:], EXP)
                            # row-sums: disjoint col-groups (parts 0 / 32)
                            nc.tensor.matmul(
                                s_pair[0:1, :], ones_col[:], e0[:],
                                start=(m == 0), stop=(m == 7))
                            nc.tensor.matmul(
                                s_pair[64:65, :], ones_col[:], e1[:],
                                start=(m == 0), stop=(m == 7))
                            a0 = asb.tile([P, C], BF16, tag="att")
                            a1 = asb.tile([P, C], BF16, tag="att")
                            nc.vector.tensor_tensor(
                                a0[:], e0[:], DT[b][m][:, nc_sl], MULT)
                            nc.vector.tensor_tensor(
                                a1[:], e1[:], DT[b][m][:, nc_sl], MULT)
                            # AV col-packed: parts 0-63 / 64-127
                            nc.tensor.matmul(
                                x_pair[0:DH, :],
                                VP[b * 8 + m][:, 2 * hp * DH:(2 * hp + 1) * DH],
                                a0[:], start=(m == 0), stop=(m == 7))
                            nc.tensor.matmul(
                                x_pair[DH:P, :],
                                VP[b * 8 + m][:, (2 * hp + 1) * DH:(2 * hp + 2) * DH],
                                a1[:], start=(m == 0), stop=(m == 7))
                        # normalize pair
                        r0_ = nsb.tile([1, C], F32, tag="r0")
                        r1_ = nsb.tile([1, C], F32, tag="r1")
                        nc.vector.reciprocal(r0_[:], s_pair[0:1, :])
                        nc.vector.reciprocal(r1_[:], s_pair[64:65, :])
                        rr0 = nsb.tile([1, C], F32R, tag="rr0")
                        rr1 = nsb.tile([1, C], F32R, tag="rr1")
                        nc.vector.tensor_copy(rr0[:], r0_[:])
                        nc.vector.tensor_copy(rr1[:], r1_[:])
                        rb0 = att_ps.tile([DH, C], F32, tag="st", bufs=4)
                        rb1 = att_ps.tile([DH, C], F32, tag="st", bufs=4)
                        nc.tensor.matmul(rb0[:], ones64r[:], rr0[:],
                                         start=True, stop=True)
                        nc.tensor.matmul(rb1[:], ones64r[:], rr1[:],
                                         start=True, stop=True)
                        rb_sb = nsb.tile([P, C], F32, tag="rb")
                        nc.scalar.copy(rb_sb[0:DH, :], rb0[:])
                        nc.scalar.copy(rb_sb[DH:P, :], rb1[:])
                        nc.vector.tensor_tensor(
                            XT[ft][:, ns0:ns0 + C], x_pair[:], rb_sb[:], MULT)

            # ---- output projection ----
            for t in range(16):
                pp = att_ps.tile([P, E], F32, tag="st", bufs=4)
                for e in range(4):
                    nc.tensor.matmul(
                        pp[:], XT[e][:, t * P:(t + 1) * P], WT[3][e][:],
                        start=(e == 0), stop=(e == 3))
                if with_bias:
                    nc.tensor.matmul(
                        pp[:], ones_row[:, 0:P], b_bf[3][:],
                        start=False, stop=True, skip_group_check=True)
                osb = asb.tile([P, E], F32, tag="osb")
                if t % 2:
                    nc.scalar.copy(osb[:], pp[:])
                else:
                    nc.vector.tensor_copy(osb[:], pp[:])
                nc.sync.dma_start(dout.ap()[t * P:(t + 1) * P, :], osb[:])


def _get_nc(with_bias=True):
    key = f"nc{int(with_bias)}"
    if key not in _CACHE:
        _CACHE[key] = _build_nc(with_bias)
    return _CACHE[key]


def _shard(inputs):
    q, k, v, d = (np.asarray(inputs[s], np.float32) for s in "qkvd")
    scale = 1.0 / math.sqrt(DH)
    Wq = np.asarray(inputs["Wq"], np.float32) * scale
    bq = np.asarray(inputs["bq"], np.float32) * scale
    Ws = [Wq, np.asarray(inputs["Wk"], np.float32),
          np.asarray(inputs["Wv"], np.float32),
          np.asarray(inputs["Wp"], np.float32)]
    bs = [bq, np.asarray(inputs["bk"], np.float32),
          np.asarray(inputs["bv"], np.float32),
          np.asarray(inputs["bp"], np.float32)]
    in_maps = []
    for c in range(NCORES):
        sl = slice(c * BLOC, (c + 1) * BLOC)
        m = {
            "q": np.ascontiguousarray(q[sl].reshape(NT, E)),
            "k": np.ascontiguousarray(k[sl].reshape(NT, E)),
            "v": np.ascontiguousarray(v[sl].reshape(NT, E)),
            "d": np.ascontiguousarray(d[sl].reshape(NT, N)),
        }
        for i, s in enumerate("qkvp"):
            m[f"W{s}"] = np.ascontiguousarray(Ws[i])
            m[f"b{s}"] = np.ascontiguousarray(bs[i].reshape(1, E))
        in_maps.append(m)
    return in_maps


def _biases_zero(inputs):
    return all(
        not np.any(np.asarray(inputs[f"b{s}"])) for s in "qkvp")


def _get_exec(with_bias):
    """Build (once) a sharded jitted callable over the 8 axon devices."""
    key = f"exec{int(with_bias)}"
    if key in _CACHE:
        return _CACHE[key]
    import jax
    from jax.sharding import Mesh, NamedSharding, PartitionSpec
    from jax.experimental.shard_map import shard_map
    from concourse import bass2jax

    nc = _get_nc(with_bias)
    bass2jax.install_neuronx_cc_hook()

    partition_name = (nc.partition_id_tensor.name
                      if nc.partition_id_tensor else None)
    in_names, out_names, out_avals, zero_outs = [], [], [], []
    for alloc in nc.m.functions[0].allocations:
        if not isinstance(alloc, mybir.MemoryLocationSet):
            continue
        name = alloc.memorylocations[0].name
        if alloc.kind == "ExternalInput":
            if name != partition_name:
                in_names.append(name)
        elif alloc.kind == "ExternalOutput":
            out_names.append(name)
            shape = tuple(alloc.tensor_shape)
            dtype = mybir.dt.np(alloc.dtype)
            out_avals.append(jax.core.ShapedArray(shape, dtype))
            zero_outs.append(np.zeros(shape, dtype))
    n_params = len(in_names)
    all_names = in_names + out_names
    if partition_name is not None:
        all_names = all_names + [partition_name]

    def _body(*args):
        operands = list(args)
        if partition_name is not None:
            operands.append(bass2jax.partition_id_tensor())
        outs = bass2jax._bass_exec_p.bind(
            *operands,
            out_avals=tuple(out_avals),
            in_names=tuple(all_names),
            out_names=tuple(out_names),
            lowering_input_output_aliases=(),
            sim_require_finite=True,
            sim_require_nnan=True,
            nc=nc,
        )
        return tuple(outs)

    devices = jax.devices()[:NCORES]
    mesh = Mesh(np.asarray(devices), ("core",))
    nspec = (PartitionSpec("core"),)
    fn = jax.jit(
        shard_map(_body, mesh=mesh,
                  in_specs=nspec * (n_params + len(out_names)),
                  out_specs=nspec * len(out_names), check_rep=False),
        keep_unused=True)
    sharding = NamedSharding(mesh, PartitionSpec("core"))
    _CACHE[key] = (fn, in_names, out_names, out_avals, zero_outs, sharding)
    return _CACHE[key]


def _concat_args(in_maps, ex):
    fn, in_names, out_names, out_avals, zero_outs, _ = ex
    concat_in = [
        np.concatenate([in_maps[c][nm] for c in range(NCORES)], axis=0)
        for nm in in_names]
    concat_zero = [
        np.zeros((NCORES * z.shape[0], *z.shape[1:]), z.dtype)
        for z in zero_outs]
    return concat_in + concat_zero


def kernel(**inputs):
    with_bias = not _biases_zero(inputs)
    ex = _get_exec(with_bias)
    fn, in_names, out_names, out_avals, zero_outs, _ = ex
    args = _concat_args(_shard(inputs), ex)
    out_arrs = fn(*args)
    out = np.asarray(out_arrs[out_names.index("out")])
    return out.reshape(B, N, E)


def bench(inputs, iters=10):
    """Time repeated executions with device-resident inputs; returns secs."""
    import time
    import jax
    with_bias = not _biases_zero(inputs)
    ex = _get_exec(with_bias)
    fn, in_names, out_names, out_avals, zero_outs, sharding = ex
    args = _concat_args(_shard(inputs), ex)
    dev_args = [jax.device_put(a, sharding) for a in args]
    jax.block_until_ready(dev_args)
    out = fn(*dev_args)
    jax.block_until_ready(out)
    times = []
    for _ in range(iters):
        t0 = time.perf_counter()
        out = fn(*dev_args)
        jax.block_until_ready(out)
        times.append(time.perf_counter() - t0)
    return times


# revision 20
# speedup vs baseline: 53.7077x; 1.0020x over previous
"""Trainium2 Bass kernel for nn_Attention_65541200937161 (sparse_attention).

Computation (per batch b, head h; B=16, N=1024, E=512, H=8, DH=64):
    qh = (q @ Wq.T + bq) split heads;  kh, vh same
    att = softmax(qh @ kh.T / sqrt(DH) + d) * d
    out = (att @ vh merged heads) @ Wp.T + bp

Sharding: data-parallel over batch B across 8 cores (2 batches/core).

Per-core layout (transposed-scores, head-paired):
    - inputs/weights cast to bf16 (DVE) then transposed on PE; evac at PSUM
    - QT/KT [128f, 2048t] bf16: head-pair 2ft,2ft+1 at partitions 0/64
    - attention loop over (batch, head-pair, n-half): K=64 QK matmuls for
      the two heads sit on disjoint PE row-groups (concurrent); +d via
      identity-matmul accumulate; exp on ACT (PSUM->SBUF, bf16);
      row-sums via ones-column matmuls into a shared [33,512] psum tile at
      partitions 0/32 (disjoint col-groups, concurrent); att''=e*dT on DVE;
      AV col-packed into x_pair [128,512] at partitions 0/64 (concurrent)
    - normalize by 1/rowsum: DVE reciprocal, f32r K=1 broadcast matmul,
      one DVE multiply into XT (bf16)
    - out = XT.T @ WpT + bp, natural layout, fp32 DMA out
The scale 1/sqrt(DH) is folded into Wq/bq on the host.
"""

import math
import os

import numpy as np

import concourse.bass as bass
import concourse.tile as tile
from concourse import bacc, mybir
from concourse.masks import make_identity

P = 128
E = 512
N = 1024
H = 8
DH = 64
B = 16
NCORES = 8
BLOC = B // NCORES          # 2 batches per core
NT = BLOC * N               # 2048 tokens per core

F32 = mybir.dt.float32
F32R = mybir.dt.float32r
BF16 = mybir.dt.bfloat16
EXP = mybir.ActivationFunctionType.Exp
MULT = mybir.AluOpType.mult

_CACHE = {}


def _build_nc(with_bias):
    repeat = int(os.environ.get("KERNEL_REPEAT", "1"))
    nc = bacc.Bacc("TRN2", target_bir_lowering=False, debug=False,
                   num_devices=1)

    dq = nc.dram_tensor("q", [NT, E], F32, kind="ExternalInput")
    dk = nc.dram_tensor("k", [NT, E], F32, kind="ExternalInput")
    dv = nc.dram_tensor("v", [NT, E], F32, kind="ExternalInput")
    dd = nc.dram_tensor("d", [NT, N], F32, kind="ExternalInput")
    dW = [nc.dram_tensor(f"W{s}", [E, E], F32, kind="ExternalInput")
          for s in "qkvp"]
    db = [nc.dram_tensor(f"b{s}", [1, E], F32, kind="ExternalInput")
          for s in "qkvp"]
    dout = nc.dram_tensor("out", [NT, E], F32, kind="ExternalOutput")

    with tile.TileContext(nc) as tc:
        for _ in range(repeat):
            _emit(nc, tc, dq, dk, dv, dd, dW, db, dout, with_bias)
    nc.compile()
    return nc


def _emit(nc, tc, dq, dk, dv, dd, dW, db, dout, with_bias):
    from contextlib import ExitStack

    with ExitStack() as ctx:
        const = ctx.enter_context(tc.tile_pool(name="const", bufs=1))
        persist = ctx.enter_context(tc.tile_pool(name="persist", bufs=1))

        # ---- constants ----
        ident = const.tile([P, P], F32, tag="ident")
        make_identity(nc, ident[:])
        ident_bf = const.tile([P, P], BF16, tag="ident_bf")
        nc.vector.tensor_copy(ident_bf[:], ident[:])
        ones_col = const.tile([P, 1], BF16, tag="ones_col")
        nc.vector.memset(ones_col[:], 1.0)
        ones64f = const.tile([1, DH], F32, tag="ones64f")
        nc.vector.memset(ones64f[:], 1.0)
        ones64r = const.tile([1, DH], F32R, tag="ones64r")
        nc.vector.tensor_copy(ones64r[:], ones64f[:])

        b_bf = []
        if with_bias:
            ones_row = const.tile([1, E], BF16, tag="ones_row")
            nc.vector.memset(ones_row[:], 1.0)
            for i in range(4):
                braw = const.tile([1, E], F32, tag=f"braw{i}")
                nc.sync.dma_start(braw[:], db[i].ap())
                bb = const.tile([1, E], BF16, tag=f"bbf{i}")
                nc.vector.tensor_copy(bb[:], braw[:])
                b_bf.append(bb)

        # ---- persistent SBUF tensors ----
        WT = [[persist.tile([P, E], BF16, tag=f"wt{w}_{e}", name=f"wt{w}_{e}")
               for e in range(4)] for w in range(4)]
        QT = [persist.tile([P, NT], BF16, tag=f"qt{i}", name=f"qt{i}")
              for i in range(4)]
        KT = [persist.tile([P, NT], BF16, tag=f"kt{i}", name=f"kt{i}")
              for i in range(4)]
        VP = [persist.tile([P, E], BF16, tag=f"vp{i}", name=f"vp{i}")
              for i in range(16)]
        DT = [[persist.tile([P, N], BF16, tag=f"dt{b}_{m}", name=f"dt{b}_{m}")
               for m in range(8)] for b in range(BLOC)]
        XT = [persist.tile([P, NT], BF16, tag=f"xtout{i}", name=f"xtout{i}")
              for i in range(4)]

        # ================= phase W/I/D: casts, transposes, projections ======
        # fp32->bf16 cast during the SWDGE load; transposes on PE (bf16,
        # 1 cyc/row) with PSUM evac split between ACT and DVE.
        with ExitStack() as pctx:
            bstage = pctx.enter_context(tc.tile_pool(name="bstage", bufs=3))
            xtin = pctx.enter_context(tc.tile_pool(name="xtin", bufs=1))
            ptr_pool = pctx.enter_context(
                tc.tile_pool(name="ptr", bufs=2, space="PSUM"))
            pj_pool = pctx.enter_context(
                tc.tile_pool(name="pj", bufs=2, space="PSUM"))
            dtr_pool = pctx.enter_context(
                tc.tile_pool(name="dtr", bufs=4, space="PSUM"))

            def load_cast(dram_ap, name):
                bfb = bstage.tile([P, 4 * E], BF16, tag="bfb", name=name)
                nc.gpsimd.dma_start(
                    bfb[:].rearrange("p (a e) -> p a e", a=4), dram_ap)
                return bfb

            # ---- weights ----
            for w in range(4):
                wbf = load_cast(
                    dW[w].ap().rearrange("(a p) e -> p a e", p=P), "wbf")
                for e in range(4):
                    ptr = ptr_pool.tile([P, E], BF16, tag="tr")
                    for a in range(4):
                        nc.tensor.transpose(
                            ptr[:, a * P:(a + 1) * P],
                            wbf[:, a * E + e * P: a * E + (e + 1) * P],
                            ident_bf[:])
                    nc.scalar.copy(WT[w][e][:], ptr[:])

            # ---- q, k -> QT, KT ----
            xt_in = [xtin.tile([P, NT], BF16, tag=f"xtin{e}", name=f"xtin{e}")
                     for e in range(4)]
            for w, (dx, QKT) in enumerate([(dq, QT), (dk, KT)]):
                for c in range(4):
                    xbf = load_cast(
                        dx.ap()[c * 512:(c + 1) * 512, :]
                        .rearrange("(a p) e -> p a e", p=P), "xbf")
                    for e in range(4):
                        ptr = ptr_pool.tile([P, E], BF16, tag="tr")
                        for a in range(4):
                            nc.tensor.transpose(
                                ptr[:, a * P:(a + 1) * P],
                                xbf[:, a * E + e * P: a * E + (e + 1) * P],
                                ident_bf[:])
                        if e % 2:
                            nc.scalar.copy(
                                xt_in[e][:, c * E:(c + 1) * E], ptr[:])
                        else:
                            nc.vector.tensor_copy(
                                xt_in[e][:, c * E:(c + 1) * E], ptr[:])
                for ft in range(4):
                    for c in range(4):
                        pp = pj_pool.tile([P, E], F32, tag="pj")
                        for e in range(4):
                            nc.tensor.matmul(
                                pp[:],
                                WT[w][e][:, ft * P:(ft + 1) * P],
                                xt_in[e][:, c * E:(c + 1) * E],
                                start=(e == 0), stop=(e == 3))
                        if with_bias:
                            nc.tensor.matmul(
                                pp[:], b_bf[w][:, ft * P:(ft + 1) * P],
                                ones_row[:], start=False, stop=True,
                                skip_group_check=True)
                        nc.vector.tensor_copy(
                            QKT[ft][:, c * E:(c + 1) * E], pp[:])

            # ---- d -> DT (transposed, bf16) ----
            for b in range(BLOC):
                for c in range(4):
                    r0 = b * N + c * 256
                    dbf = load_cast(
                        dd.ap()[r0:r0 + 256, :]
                        .rearrange("(a p) m -> p a m", p=P), "dbf")
                    for m in range(8):
                        ptr = dtr_pool.tile([P, 256], BF16, tag="dtr")
                        for a in range(2):
                            nc.tensor.transpose(
                                ptr[:, a * P:(a + 1) * P],
                                dbf[:, a * N + m * P: a * N + (m + 1) * P],
                                ident_bf[:])
                        if m % 2:
                            nc.scalar.copy(
                                DT[b][m][:, c * 256:(c + 1) * 256], ptr[:])
                        else:
                            nc.vector.tensor_copy(
                                DT[b][m][:, c * 256:(c + 1) * 256], ptr[:])

            # ---- v -> VP ----
            for c in range(4):
                xbf = load_cast(
                    dv.ap()[c * 512:(c + 1) * 512, :]
                    .rearrange("(a p) e -> p a e", p=P), "xbf")
                for e in range(4):
                    ptr = ptr_pool.tile([P, E], BF16, tag="tr")
                    for a in range(4):
                        nc.tensor.transpose(
                            ptr[:, a * P:(a + 1) * P],
                            xbf[:, a * E + e * P: a * E + (e + 1) * P],
                            ident_bf[:])
                    if e % 2:
                        nc.scalar.copy(
                            xt_in[e][:, c * E:(c + 1) * E], ptr[:])
                    else:
                        nc.vector.tensor_copy(
                            xt_in[e][:, c * E:(c + 1) * E], ptr[:])
            for t in range(16):
                pp = pj_pool.tile([P, E], F32, tag="pj")
                for e in range(4):
                    nc.tensor.matmul(
                        pp[:],
                        xt_in[e][:, t * P:(t + 1) * P],
                        WT[2][e][:],
                        start=(e == 0), stop=(e == 3))
                if with_bias:
                    nc.tensor.matmul(
                        pp[:], ones_row[:, 0:P], b_bf[2][:],
                        start=False, stop=True, skip_group_check=True)
                if t % 2:
                    nc.scalar.copy(VP[t][:], pp[:])
                else:
                    nc.vector.tensor_copy(VP[t][:], pp[:])

        # ================= phase A: attention ==============================
        with ExitStack() as actx:
            att_ps = actx.enter_context(
                tc.tile_pool(name="attps", bufs=1, space="PSUM"))
            asb = actx.enter_context(tc.tile_pool(name="asb", bufs=3))
            nsb = actx.enter_context(tc.tile_pool(name="nsb", bufs=2))

            C = 512   # n-chunk
            for b in range(BLOC):
                for hp in range(4):
                    ft = hp
                    for nh in range(2):
                        ns0 = b * N + nh * C
                        s_pair = att_ps.tile([65, C], F32, tag="s", bufs=2)
                        x_pair = att_ps.tile([P, C], F32, tag="x", bufs=2)
                        for m in range(8):
                            mt = slice(b * N + m * P, b * N + (m + 1) * P)
                            nc_sl = slice(nh * C, (nh + 1) * C)
                            st0 = att_ps.tile([P, C], F32, tag="st", bufs=4)
                            st1 = att_ps.tile([P, C], F32, tag="st", bufs=4)
                            # QK for both heads: disjoint PE row-groups
                            nc.tensor.matmul(
                                st0[:], KT[ft][0:DH, mt],
                                QT[ft][0:DH, ns0:ns0 + C],
                                start=True, stop=True)
                            nc.tensor.matmul(
                                st1[:], KT[ft][DH:P, mt],
                                QT[ft][DH:P, ns0:ns0 + C],
                                start=True, stop=True)
                            # + d (identity matmul accumulate)
                            nc.tensor.matmul(
                                st0[:], ident_bf[:], DT[b][m][:, nc_sl],
                                start=False, stop=True, skip_group_check=True)
                            nc.tensor.matmul(
                                st1[:], ident_bf[:], DT[b][m][:, nc_sl],
                                start=False, stop=True, skip_group_check=True)
                            e0 = asb.tile([P, C], BF16, tag="e")
                            e1 = asb.tile([P, C], BF16, tag="e")
                            nc.scalar.activation(e0[:], st0[:], EXP)
                            nc.scalar.activation(e1[:], st1[:], EXP)
                            # row-sums: disjoint col-groups (parts 0 / 32)
                            nc.tensor.matmul(
                                s_pair[0:1, :], ones_col[:], e0[:],
                                start=(m == 0), stop=(m == 7))
                            nc.tensor.matmul(
                                s_pair[64:65, :], ones_col[:], e1[:],
                                start=(m == 0), stop=(m == 7))
                            a0 = asb.tile([P, C], BF16, tag="att")
                            a1 = asb.tile([P, C], BF16, tag="att")
                            nc.vector.tensor_tensor(
                                a0[:], e0[:], DT[b][m][:, nc_sl], MULT)
                            nc.vector.tensor_tensor(
                                a1[:], e1[:], DT[b][m][:, nc_sl], MULT)
                            # AV col-packed: parts 0-63 / 64-127
                            nc.tensor.matmul(
                                x_pair[0:DH, :],
                                VP[b * 8 + m][:, 2 * hp * DH:(2 * hp + 1) * DH],
                                a0[:], start=(m == 0), stop=(m == 7))
                            nc.tensor.matmul(
                                x_pair[DH:P, :],
                                VP[b * 8 + m][:, (2 * hp + 1) * DH:(2 * hp + 2) * DH],
                                a1[:], start=(m == 0), stop=(m == 7))
                        # normalize pair
                        rr0 = nsb.tile([1, C], F32R, tag="rr0")
                        rr1 = nsb.tile([1, C], F32R, tag="rr1")
                        with nc.allow_low_precision(
                                reason="1/rowsum rounded to f32r (~1e-4)"):
                            nc.vector.reciprocal(rr0[:], s_pair[0:1, :])
                            nc.vector.reciprocal(rr1[:], s_pair[64:65, :])
                        rb0 = att_ps.tile([DH, C], F32, tag="x", bufs=2)
                        rb1 = att_ps.tile([DH, C], F32, tag="x", bufs=2)
                        nc.tensor.matmul(rb0[:], ones64r[:], rr0[:],
                                         start=True, stop=True)
                        nc.tensor.matmul(rb1[:], ones64r[:], rr1[:],
                                         start=True, stop=True)
                        rb_sb = nsb.tile([P, C], F32, tag="rb")
                        nc.scalar.copy(rb_sb[0:DH, :], rb0[:])
                        nc.scalar.copy(rb_sb[DH:P, :], rb1[:])
                        nc.vector.tensor_tensor(
                            XT[ft][:, ns0:ns0 + C], x_pair[:], rb_sb[:], MULT)

            # ---- output projection ----
            for t in range(16):
                pp = att_ps.tile([P, E], F32, tag="st", bufs=4)
                for e in range(4):
                    nc.tensor.matmul(
                        pp[:], XT[e][:, t * P:(t + 1) * P], WT[3][e][:],
                        start=(e == 0), stop=(e == 3))
                if with_bias:
                    nc.tensor.matmul(
                        pp[:], ones_row[:, 0:P], b_bf[3][:],
                        start=False, stop=True, skip_group_check=True)
                osb = asb.tile([P, E], F32, tag="osb")
                if t % 2:
                    nc.scalar.copy(osb[:], pp[:])
                else:
                    nc.vector.tensor_copy(osb[:], pp[:])
                nc.sync.dma_start(dout.ap()[t * P:(t + 1) * P, :], osb[:])


def _get_nc(with_bias=True):
    key = f"nc{int(with_bias)}"
    if key not in _CACHE:
        _CACHE[key] = _build_nc(with_bias)
    return _CACHE[key]


def _shard(inputs):
    q, k, v, d = (np.asarray(inputs[s], np.float32) for s in "qkvd")
    scale = 1.0 / math.sqrt(DH)
    Wq = np.asarray(inputs["Wq"], np.float32) * scale
    bq = np.asarray(inputs["bq"], np.float32) * scale
    Ws = [Wq, np.asarray(inputs["Wk"], np.float32),
          np.asarray(inputs["Wv"], np.float32),
          np.asarray(inputs["Wp"], np.float32)]
    bs = [bq, np.asarray(inputs["bk"], np.float32),
          np.asarray(inputs["bv"], np.float32),
          np.asarray(inputs["bp"], np.float32)]
    in_maps = []
    for c in range(NCORES):
        sl = slice(c * BLOC, (c + 1) * BLOC)
        m = {
            "q": np.ascontiguousarray(q[sl].reshape(NT, E)),
            "k": np.ascontiguousarray(k[sl].reshape(NT, E)),
            "v": np.ascontiguousarray(v[sl].reshape(NT, E)),
            "d": np.ascontiguousarray(d[sl].reshape(NT, N)),
        }
        for i, s in enumerate("qkvp"):
            m[f"W{s}"] = np.ascontiguousarray(Ws[i])
            m[f"b{s}"] = np.ascontiguousarray(bs[i].reshape(1, E))
        in_maps.append(m)
    return in_maps


def _biases_zero(inputs):
    return all(
        not np.any(np.asarray(inputs[f"b{s}"])) for s in "qkvp")


def _get_exec(with_bias):
    """Build (once) a sharded jitted callable over the 8 axon devices."""
    key = f"exec{int(with_bias)}"
    if key in _CACHE:
        return _CACHE[key]
    import jax
    from jax.sharding import Mesh, NamedSharding, PartitionSpec
    from jax.experimental.shard_map import shard_map
    from concourse import bass2jax

    nc = _get_nc(with_bias)
    bass2jax.install_neuronx_cc_hook()

    partition_name = (nc.partition_id_tensor.name
                      if nc.partition_id_tensor else None)
    in_names, out_names, out_avals, zero_outs = [], [], [], []
    for alloc in nc.m.functions[0].allocations:
        if not isinstance(alloc, mybir.MemoryLocationSet):
            continue
        name = alloc.memorylocations[0].name
        if alloc.kind == "ExternalInput":
            if name != partition_name:
                in_names.append(name)
        elif alloc.kind == "ExternalOutput":
            out_names.append(name)
            shape = tuple(alloc.tensor_shape)
            dtype = mybir.dt.np(alloc.dtype)
            out_avals.append(jax.core.ShapedArray(shape, dtype))
            zero_outs.append(np.zeros(shape, dtype))
    n_params = len(in_names)
    all_names = in_names + out_names
    if partition_name is not None:
        all_names = all_names + [partition_name]

    def _body(*args):
        operands = list(args)
        if partition_name is not None:
            operands.append(bass2jax.partition_id_tensor())
        outs = bass2jax._bass_exec_p.bind(
            *operands,
            out_avals=tuple(out_avals),
            in_names=tuple(all_names),
            out_names=tuple(out_names),
            lowering_input_output_aliases=(),
            sim_require_finite=True,
            sim_require_nnan=True,
            nc=nc,
        )
        return tuple(outs)

    devices = jax.devices()[:NCORES]
    mesh = Mesh(np.asarray(devices), ("core",))
    nspec = (PartitionSpec("core"),)
    fn = jax.jit(
        shard_map(_body, mesh=mesh,
                  in_specs=nspec * (n_params + len(out_names)),
                  out_specs=nspec * len(out_names), check_rep=False),
        keep_unused=True)
    sharding = NamedSharding(mesh, PartitionSpec("core"))
    _CACHE[key] = (fn, in_names, out_names, out_avals, zero_outs, sharding)
    return _CACHE[key]


def _concat_args(in_maps, ex):
    fn, in_names, out_names, out_avals, zero_outs, _ = ex
    concat_in = [
        np.concatenate([in_maps[c][nm] for c in range(NCORES)], axis=0)
        for nm in in_names]
    concat_zero = [
        np.zeros((NCORES * z.shape[0], *z.shape[1:]), z.dtype)
        for z in zero_outs]
    return concat_in + concat_zero


def kernel(**inputs):
    with_bias = not _biases_zero(inputs)
    ex = _get_exec(with_bias)
    fn, in_names, out_names, out_avals, zero_outs, _ = ex
    args = _concat_args(_shard(inputs), ex)
    out_arrs = fn(*args)
    out = np.asarray(out_arrs[out_names.index("out")])
    return out.reshape(B, N, E)


def bench(inputs, iters=10):
    """Time repeated executions with device-resident inputs; returns secs."""
    import time
    import jax
    with_bias = not _biases_zero(inputs)
    ex = _get_exec(with_bias)
    fn, in_names, out_names, out_avals, zero_outs, sharding = ex
    args = _concat_args(_shard(inputs), ex)
    dev_args = [jax.device_put(a, sharding) for a in args]
    jax.block_until_ready(dev_args)
    out = fn(*dev_args)
    jax.block_until_ready(out)
    times = []
    for _ in range(iters):
        t0 = time.perf_counter()
        out = fn(*dev_args)
        jax.block_until_ready(out)
        times.append(time.perf_counter() - t0)
    return times
